# revision 31
# baseline (speedup 1.0000x reference)
"""Trainium2 Bass kernel for nn_Attention_53687091200195.

Reference computation (per batch b):
    Q = relu(x @ Wq + bq); K = relu(x @ Wk + bk); V = relu(x @ Wv + bv)
    S = Q @ K^T / sqrt(64); P = softmax(S, axis=-1); out = P @ V

Shapes: x [16, 2048, 64] f32, W* [64, 128] f32, b* [128] f32 -> out [16, 2048, 128].

Sharding: data-parallel over batch. 8 cores x 2 batches each; weights replicated.

Measured TRN2 facts this design is built on (from NTFF traces of prior
versions): the PE streams 1 moving-column/cycle at 2.4GHz for EVERY dtype --
fp8 DoubleRow only fuses instruction pairs and is net slower per column, so
bf16/fp32r at 1 col/cycle is optimal; a 512-col matmul slot is ~245ns
end-to-end.  ACT exp [128,1024] with bf16 out is ~1110ns; DVE bf16 all-SBUF
adds run ~0.55ns/elem; GPSIMD cannot touch PSUM and its adds are ~2ns/elem.
Engine semaphores are COUNTERS, so any PE wait on engine X transitively
waits for everything queued earlier on X -- cross-engine dependencies must
only point at work that is already drained, or they head-of-line block the
PE.  Exp relief on DVE always lost to this effect; exp is all-ACT.

Per-core design (SPMD, identical program):
  - Token-permuted layout: internal token n~ = j*128 + p maps to real token
    p*16 + j; attention is permutation-equivariant, and this makes the
    x-load / out-store DMAs contiguous per partition.
  - All-bf16 operand path: xT [65, 2048] (bias folded as an all-ones row),
    Q^T/K^T [128d, 2048], V [key, m, d], E [key, m, n] all bf16 (PE speed is
    the same as fp32r; the 2-byte tiles halve SBUF traffic and enable the
    DVE esum below).  End-to-end rel err ~7e-3 vs the 2e-2 gate.
  - The softmax denominator is NOT a third PE stream (that costs ~31us):
    esum[p, n] = sum_m E[p, m, n] is built as a 12+4 split add-tree --
    GPSIMD pair-folds m0-7 early (two 2048-elem bf16 adds), DVE folds m8-11
    mid-window and only m12-15 after the last exp -- then one [1, n]
    ones-matmul pair + transposed reciprocal give 1/den.  Per-element tree
    rounding averages out 1/sqrt(128) in the partition sum.
  - Cross-chunk software pipelining: each (batch, chunk) window runs a
    30-iteration schedule (scores+exp 0-15 with the exp two st-ring slots
    behind, PV lag 2, esum folds at 5/9/13/14/15/17/18/19, den matmul 20,
    den_t + reciprocal and the out-transposes from iter 24 (21 for the last
    window), normalize fused into each PSUM->SBUF copy one iter later,
    stores at +4/+5), and windows start every 20 iterations so one window's
    epilogue always hides under the next window's score stream.  All tail
    instructions are scheduled late enough that their cross-engine inputs
    are already drained when the in-order PE reaches them.
  - PSUM banks (8): score ring 2x(2 banks), PV accumulator 2, den [1,512] 1,
    and one scratch bank holding all 8 output-transpose slots (bf16) with
    den_t borrowing slot 7 under subtile-WAR protection.
"""

import numpy as np

import concourse.bass as bass
import concourse.mybir as mybir
import concourse.tile as tile
from concourse import bacc
from concourse.bass_utils import run_bass_kernel_spmd

N_CORES = 8
B_PER_CORE = 2
N_TOK = 2048
C_IN = 64
D = 128
P = 128
N_TILES = N_TOK // P          # 16
N_CHUNK = 1024
N_CHUNKS = N_TOK // N_CHUNK   # 2
JT = N_CHUNK // P             # 8
SCALE = 1.0 / 8.0             # 1/sqrt(64)

F32 = mybir.dt.float32
BF16 = mybir.dt.bfloat16
I16 = mybir.dt.int16

# Schraudolph constants for exp(S/8) as bf16 bits via int16:
# bits = S * (128*log2e/8) + 128*127 + corrections (+0.5 truncating
# convert, -5.51 centers the piecewise-linear 2^frac sawtooth).
SCH_A = 23.083120654223414
SCH_B = 16256.0 + 0.5 - 0.5 * 128.0 * 0.0861

# exp engine per m (A=ACT exact, D=DVE schraudolph)
EXP_PATTERN = "AAAAAAAAAAAAAAAA"
# relu engine for the 8 projection slices
RELU_PATTERN = "ADADADAD"

WLEN = 30      # window schedule length (iters)
STRIDE = 20    # window start spacing


def build_program():
    nc = bacc.Bacc("TRN2", target_bir_lowering=False, debug=False,
                   num_devices=N_CORES)

    x = nc.dram_tensor("x", [B_PER_CORE, N_TOK, C_IN], F32, kind="ExternalInput").ap()
    wq = nc.dram_tensor("Wq", [C_IN, D], F32, kind="ExternalInput").ap()
    bq = nc.dram_tensor("bq", [D], F32, kind="ExternalInput").ap()
    wk = nc.dram_tensor("Wk", [C_IN, D], F32, kind="ExternalInput").ap()
    bk = nc.dram_tensor("bk", [D], F32, kind="ExternalInput").ap()
    wv = nc.dram_tensor("Wv", [C_IN, D], F32, kind="ExternalInput").ap()
    bv = nc.dram_tensor("bv", [D], F32, kind="ExternalInput").ap()
    out = nc.dram_tensor("out", [B_PER_CORE, N_TOK, D], F32, kind="ExternalOutput").ap()

    with tile.TileContext(nc) as tc:
        kernel_body(tc, out, x, (wq, bq), (wk, bk), (wv, bv))

    nc.compile()
    return nc


def kernel_body(tc, out, x, qw, kw, vw):
    nc = tc.nc
    from contextlib import ExitStack
    ctx = ExitStack()
    with ctx:
        consts = ctx.enter_context(tc.tile_pool(name="consts", bufs=1))
        perb = ctx.enter_context(tc.tile_pool(name="perb", bufs=2))
        epool = ctx.enter_context(tc.tile_pool(name="epool", bufs=1))
        ep = ctx.enter_context(tc.tile_pool(name="ep", bufs=2))

        # x first: both batches' quarter-DMAs issue before anything else
        x_nat2 = consts.tile([P, N_TILES, B_PER_CORE, C_IN], F32, name="x_nat2",
                             tag="x_nat2")
        H = N_TILES // 4
        for jh in range(4):
            for bb in range(B_PER_CORE):
                eng = nc.sync if bb == 0 else (nc.scalar if jh < 2 else nc.gpsimd)
                eng.dma_start(
                    out=x_nat2[:, jh * H:(jh + 1) * H, bb, :],
                    in_=bass.AP(
                        tensor=x.tensor,
                        offset=bb * N_TOK * C_IN + jh * H * C_IN,
                        ap=[[N_TILES * C_IN, P], [C_IN, H], [1, C_IN]],
                    ),
                )

        # --- constants ---
        identity = consts.tile([P, P], F32)
        nc.vector.memset(identity[:], 0.0)
        nc.gpsimd.affine_select(
            out=identity[:], in_=identity[:],
            compare_op=mybir.AluOpType.not_equal, fill=1.0,
            base=0, pattern=[[-1, P]], channel_multiplier=1)
        id16 = consts.tile([P, P], BF16)
        nc.vector.tensor_copy(out=id16[:], in_=identity[:])
        ones16 = consts.tile([P, 16], BF16)
        nc.vector.memset(ones16[:], 1.0)

        # Bias-folded weights in bf16: w2[c, d], c=64 row is the bias.
        w_sb = {}
        for name, (w, b) in (("q", qw), ("k", kw), ("v", vw)):
            wf = consts.tile([C_IN, D], F32, name=f"wf_{name}", tag=f"wf_{name}")
            nc.sync.dma_start(out=wf[:], in_=w[:])
            bf = consts.tile([1, D], F32, name=f"bf_{name}", tag=f"bf_{name}")
            nc.sync.dma_start(out=bf[:], in_=b[:])
            w2 = consts.tile([C_IN + 1, D], BF16, name=f"w_{name}",
                             tag=f"w_{name}")
            nc.vector.tensor_copy(out=w2[0:C_IN, :], in_=wf[:])
            nc.vector.tensor_copy(out=w2[C_IN:C_IN + 1, :], in_=bf[:])
            w_sb[name] = w2

        xTs = [perb.tile([C_IN + 1, N_TOK], BF16, name=f"xT_{bb}",
                         tag=f"xT_{bb}", bufs=1)
               for bb in range(B_PER_CORE)]
        for bb in range(B_PER_CORE):
            nc.gpsimd.memset(xTs[bb][C_IN:C_IN + 1, :], 1.0)

        qTs = [perb.tile([D, N_TOK], BF16, name=f"qT_{bb}",
                         tag=f"qT_{bb}", bufs=1) for bb in range(B_PER_CORE)]
        kTs = [perb.tile([D, N_TOK], BF16, name=f"kT_{bb}",
                         tag=f"kT_{bb}", bufs=1) for bb in range(B_PER_CORE)]
        v_sbs = [perb.tile([P, N_TILES, D], BF16, name=f"v_sb_{bb}",
                           tag=f"v_sb_{bb}", bufs=1) for bb in range(B_PER_CORE)]

        relu_iter = list(RELU_PATTERN)

        # ---------------- Phase B defs (pools filled in later) ----------------
        pools = {}
        e_all = epool.tile([P, N_TILES, N_CHUNK], BF16, tag="e_all",
                           name="e_all")
        tmpA = epool.tile([P, 4, N_CHUNK], BF16, tag="tmpA", name="tmpA")
        tmpB = epool.tile([P, 4, N_CHUNK], BF16, tag="tmpB", name="tmpB")
        exp_engs = list(EXP_PATTERN)

        windows = [(b, c) for b in range(B_PER_CORE) for c in range(N_CHUNKS)]
        state = {}

        def emit(wi, t):
            b, chunk = windows[wi]
            last = wi == len(windows) - 1
            DEN = 20                    # den matmul iter
            TAIL = 21 if last else 24   # first out-transpose iter
            qT, kT, v_sb = qTs[b], kTs[b], v_sbs[b]
            n0 = chunk * N_CHUNK
            if t == 0:
                state[wi] = {
                    "outu": ep.tile([P, N_CHUNK], BF16, tag="outu",
                                    name=f"outu_{wi}"),
                    "o_sb": ep.tile([P, JT, D], F32, tag="o_sb",
                                    name=f"o_sb_{wi}"),
                    "den_sb": ep.tile([1, N_CHUNK], BF16, tag="den_sb",
                                      name=f"den_sb_{wi}"),
                    "recip": ep.tile([P, JT], F32, tag="recip",
                                     name=f"recip_{wi}"),
                }
            st_ = state[wi]

            if t < N_TILES:
                m = t
                st = pools["pst"].tile([P, N_CHUNK], F32, tag="st",
                                       name=f"st_{wi}_{m}")
                for h in range(2):
                    nc.tensor.matmul(
                        st[:, h * 512:(h + 1) * 512],
                        kT[:, m * P:(m + 1) * P],
                        qT[:, n0 + h * 512:n0 + (h + 1) * 512],
                        start=True, stop=True)
                if exp_engs[m] == "A":
                    nc.scalar.activation(
                        out=e_all[:, m, :], in_=st[:],
                        func=mybir.ActivationFunctionType.Exp, scale=SCALE)
                else:
                    nc.vector.tensor_scalar(
                        out=e_all[:, m, :].bitcast(I16), in0=st[:],
                        scalar1=SCH_A, scalar2=SCH_B,
                        op0=mybir.AluOpType.mult,
                        op1=mybir.AluOpType.add)
            # PV lag 2 (window 0 catches up m=0..6 at t=8, after the
            # prologue's PSUM pools have closed and pacc exists)
            pv_ms = []
            if wi == 0:
                if t == 8:
                    pv_ms = list(range(0, 7))
                elif 9 <= t < N_TILES + 2:
                    pv_ms = [t - 2]
            elif 2 <= t < N_TILES + 2:
                pv_ms = [t - 2]
            for m in pv_ms:
                if m == 0:
                    st_["acc"] = pools["pacc"].tile(
                        [P, N_CHUNK], F32, tag="acc", name=f"acc_{wi}")
                for h in range(2):
                    nc.tensor.matmul(
                        st_["acc"][:, h * 512:(h + 1) * 512],
                        v_sb[:, m, :],
                        e_all[:, m, h * 512:(h + 1) * 512],
                        start=(m == 0), stop=(m == N_TILES - 1))
                if m == N_TILES - 1:
                    nc.vector.tensor_copy(out=st_["outu"][:], in_=st_["acc"][:])
            # esum half-trees (bf16, all-SBUF). A-half (m 0-7) runs early:
            # gpsimd does two 2048-elem pair-adds, DVE folds; B-half after
            # the last exp on DVE.
            if t == 5:
                nc.gpsimd.tensor_tensor(
                    out=tmpA[:, 0:2, :], in0=e_all[:, 0:2, :],
                    in1=e_all[:, 2:4, :], op=mybir.AluOpType.add)
            if t == 9:
                nc.gpsimd.tensor_tensor(
                    out=tmpA[:, 2:4, :], in0=e_all[:, 4:6, :],
                    in1=e_all[:, 6:8, :], op=mybir.AluOpType.add)
            if t == 13:
                # fold m8-11 into tmpB[2:4], collapse tmpA (m0-7)
                nc.vector.tensor_tensor(
                    out=tmpB[:, 2:4, :], in0=e_all[:, 8:10, :],
                    in1=e_all[:, 10:12, :], op=mybir.AluOpType.add)
                nc.vector.tensor_tensor(
                    out=tmpA[:, 0:2, :], in0=tmpA[:, 0:2, :],
                    in1=tmpA[:, 2:4, :], op=mybir.AluOpType.add)
            if t == 14:
                nc.vector.tensor_tensor(
                    out=tmpA[:, 0:1, :], in0=tmpA[:, 0:1, :],
                    in1=tmpA[:, 1:2, :], op=mybir.AluOpType.add)
                nc.vector.tensor_tensor(
                    out=tmpB[:, 2:3, :], in0=tmpB[:, 2:3, :],
                    in1=tmpB[:, 3:4, :], op=mybir.AluOpType.add)
            if t == 15:
                # esum(m0-11) ready mid-window
                nc.vector.tensor_tensor(
                    out=tmpA[:, 0:1, :], in0=tmpA[:, 0:1, :],
                    in1=tmpB[:, 2:3, :], op=mybir.AluOpType.add)
            if t == 17:
                nc.vector.tensor_tensor(
                    out=tmpB[:, 0:2, :], in0=e_all[:, 12:14, :],
                    in1=e_all[:, 14:16, :], op=mybir.AluOpType.add)
            if t == 18:
                nc.vector.tensor_tensor(
                    out=tmpB[:, 0:1, :], in0=tmpB[:, 0:1, :],
                    in1=tmpB[:, 1:2, :], op=mybir.AluOpType.add)
            if t == 19:
                nc.vector.tensor_tensor(
                    out=tmpA[:, 0:1, :], in0=tmpA[:, 0:1, :],
                    in1=tmpB[:, 0:1, :], op=mybir.AluOpType.add)
            if t == DEN:
                # den = ones^T esum, h-halves into sequential 2KB tiles;
                # late enough that the esum merge is surely drained
                for h in range(2):
                    dn = pools["ptr2"].tile([1, 512], F32, tag="den",
                                            name=f"den_{wi}_{h}", bufs=1)
                    nc.tensor.matmul(
                        dn[:], ones16[:, 0:1],
                        tmpA[:, 0, h * 512:(h + 1) * 512],
                        start=True, stop=True)
                    nc.vector.tensor_copy(
                        out=st_["den_sb"][:, h * 512:(h + 1) * 512], in_=dn[:])
            if t == TAIL:
                # one scratch bank (bf16): 8 transpose slots; den_t borrows
                # slot 7 first (subtile WAR keeps tr7 off it until the
                # reciprocal has read)
                scr = pools["ptr2"].tile([P, 8 * P], BF16, tag="scr",
                                         name=f"scr_{wi}", bufs=1)
                st_["scr"] = scr
            if t == TAIL + 1:
                scr = st_["scr"]
                for jt in range(JT):
                    nc.tensor.transpose(
                        scr[:, 7 * P + 2 * jt:7 * P + 2 * jt + 1],
                        st_["den_sb"][:, jt * P:(jt + 1) * P],
                        id16[:1, :1])
                nc.vector.reciprocal(
                    out=st_["recip"][:],
                    in_=scr[:].rearrange("p (a b) -> p a b", b=2)[:, 7 * P // 2:7 * P // 2 + JT, 0])
            # out-transposes 2/iter from TAIL; norms (fused normalize in
            # the PSUM->SBUF copy, DVE) chase one iter behind the reciprocal
            if TAIL <= t < TAIL + 4:
                for jt in range((t - TAIL) * 2, (t - TAIL + 1) * 2):
                    slot = jt * P
                    scr = st_["scr"]
                    nc.tensor.transpose(scr[:, slot:slot + P],
                                        st_["outu"][:, jt * P:(jt + 1) * P],
                                        id16[:])
            if TAIL + 1 <= t < TAIL + 5:
                for jt in range((t - TAIL - 1) * 2, (t - TAIL) * 2):
                    slot = jt * P
                    scr = st_["scr"]
                    if last and jt % 2 == 1:
                        nc.scalar.activation(
                            out=st_["o_sb"][:, jt, :], in_=scr[:, slot:slot + P],
                            func=mybir.ActivationFunctionType.Copy,
                            scale=st_["recip"][:, jt:jt + 1])
                    else:
                        nc.vector.tensor_scalar(
                            out=st_["o_sb"][:, jt, :], in0=scr[:, slot:slot + P],
                            scalar1=st_["recip"][:, jt:jt + 1], scalar2=None,
                            op0=mybir.AluOpType.mult)
            if t == TAIL + 4 or t == TAIL + 5:
                half = t - TAIL - 4
                nc.sync.dma_start(
                    out=bass.AP(
                        tensor=out.tensor,
                        offset=(b * N_TOK + chunk * JT + half * JT // 2) * D,
                        ap=[[N_TILES * D, P], [D, JT // 2], [1, D]],
                    ),
                    in_=st_["o_sb"][:, half * JT // 2:(half + 1) * JT // 2, :],
                )

        # ---------------- Phase A: prologue -----------------------------
        # ppj (4 banks) outlives ptr/pvp (4 banks); once those close, the
        # score ring opens on the RIGHT side of the PSUM heap so window 0's
        # first 8 score iterations overlap the s=1 projection units.
        ppj_cm = tc.tile_pool(name="ppj", bufs=2, space="PSUM")
        ppj = ppj_cm.__enter__()
        with tc.tile_pool(name="ptr", bufs=2, space="PSUM") as ptr, \
             tc.tile_pool(name="pvp", bufs=2, space="PSUM") as pvp:

            def x_tr(q):
                xt4 = ptr.tile([P, 4, P], F32, tag="tr", name=f"xt4_{q}")
                for js in range(4):
                    j = 4 * q + js
                    nc.tensor.transpose(
                        xt4[:, js, :], x_nat2[:, j, :, :], identity[:])
                for bb in range(B_PER_CORE):
                    src = xt4[bb * C_IN:(bb + 1) * C_IN, :, :]
                    dst = xTs[bb][0:C_IN, 4 * q * P:(4 * q + 4) * P]
                    if bb == 0:
                        nc.vector.tensor_copy(
                            out=dst.rearrange("c (j t) -> c j t", j=4), in_=src)
                    else:
                        nc.scalar.copy(
                            out=dst.rearrange("c (j t) -> c j t", j=4), in_=src)

            def v_dir(bb, q):
                vp = pvp.tile([P, 4, P], F32, tag="vp", name=f"vp_{bb}_{q}")
                for js in range(4):
                    j = 4 * q + js
                    nc.tensor.matmul(
                        vp[:, js, :],
                        xTs[bb][:, j * P:(j + 1) * P],
                        w_sb["v"][:],
                        start=True, stop=True)
                dst = v_sbs[bb][:, 4 * q:4 * q + 4, :]
                if bb == 0:
                    nc.vector.tensor_scalar_max(dst, vp[:], 0.0)
                else:
                    nc.scalar.activation(
                        out=dst, in_=vp[:],
                        func=mybir.ActivationFunctionType.Relu, scale=1.0)

            def qk_unit(bb, name, s):
                # projection slice s (1024 tokens)
                t = (qTs if name == "q" else kTs)[bb]
                pj = ppj.tile([D, 2, 512], F32, tag="pj",
                              name=f"pj_{bb}_{name}_{s}")
                for h in range(2):
                    nc.tensor.matmul(
                        pj[:, h, :], w_sb[name][:],
                        xTs[bb][:, s * 1024 + h * 512:s * 1024 + (h + 1) * 512],
                        start=True, stop=True)
                eng = relu_iter.pop(0)
                dst = t[:, s * 1024:(s + 1) * 1024]
                if eng == "A":
                    nc.scalar.activation(
                        out=dst, in_=pj[:],
                        func=mybir.ActivationFunctionType.Relu, scale=1.0)
                else:
                    nc.vector.tensor_scalar_max(dst, pj[:], 0.0)

            for q in range(4):
                x_tr(q)
                if q >= 1:
                    for bb in range(B_PER_CORE):
                        v_dir(bb, q - 1)
                if q == 2:
                    for name in ("q", "k"):
                        for bb in range(B_PER_CORE):
                            qk_unit(bb, name, 0)
            for bb in range(B_PER_CORE):
                v_dir(bb, 3)
        # ptr/pvp closed; right-side score ring + window-0 head vs s=1 units
        pools["pst"] = ctx.enter_context(
            tc.tile_pool(name="pst", bufs=2, space="PSUM", side="right"))
        for i, (name, bb) in enumerate(
                (("q", 0), ("q", 1), ("k", 0), ("k", 1))):
            qk_unit(bb, name, 1)
            emit(0, 2 * i)
            emit(0, 2 * i + 1)
        ppj_cm.__exit__(None, None, None)
        pools["pacc"] = ctx.enter_context(
            tc.tile_pool(name="pacc", bufs=1, space="PSUM"))
        pools["ptr2"] = ctx.enter_context(
            tc.tile_pool(name="ptr2", bufs=2, space="PSUM"))

        starts = [-8] + [12 + STRIDE * i for i in range(len(windows) - 1)]
        total_g = starts[-1] + WLEN
        for g in range(total_g):
            for wi in range(len(windows)):
                t = g - starts[wi]
                if 0 <= t < WLEN and not (wi == 0 and t < 8):
                    emit(wi, t)


_NC_CACHE = None


def _get_program():
    global _NC_CACHE
    if _NC_CACHE is None:
        _NC_CACHE = build_program()
    return _NC_CACHE


def kernel(x, Wq, bq, Wk, bk, Wv, bv, _trace=False):
    x = np.ascontiguousarray(np.asarray(x, dtype=np.float32))
    full_b = x.shape[0]
    assert full_b == N_CORES * B_PER_CORE, x.shape
    nc = _get_program()
    common = {
        "Wq": np.ascontiguousarray(np.asarray(Wq, np.float32)),
        "bq": np.ascontiguousarray(np.asarray(bq, np.float32)),
        "Wk": np.ascontiguousarray(np.asarray(Wk, np.float32)),
        "bk": np.ascontiguousarray(np.asarray(bk, np.float32)),
        "Wv": np.ascontiguousarray(np.asarray(Wv, np.float32)),
        "bv": np.ascontiguousarray(np.asarray(bv, np.float32)),
    }
    in_maps = [
        {"x": x[c * B_PER_CORE:(c + 1) * B_PER_CORE], **common}
        for c in range(N_CORES)
    ]
    res = run_bass_kernel_spmd(nc, in_maps, list(range(N_CORES)), trace=_trace)
    outs = np.concatenate([res.results[c]["out"] for c in range(N_CORES)], axis=0)
    if _trace:
        kernel.last_exec_time_ns = res.exec_time_ns
        kernel.last_trace_info = (res.profile_json,
                                  (res.instructions_and_trace or (None, None))[1])
    return outs


# revision 32
# speedup vs baseline: 1.0264x; 1.0264x over previous
"""Trainium2 Bass kernel for nn_Attention_53687091200195.

Reference computation (per batch b):
    Q = relu(x @ Wq + bq); K = relu(x @ Wk + bk); V = relu(x @ Wv + bv)
    S = Q @ K^T / sqrt(64); P = softmax(S, axis=-1); out = P @ V

Shapes: x [16, 2048, 64] f32, W* [64, 128] f32, b* [128] f32 -> out [16, 2048, 128].

Sharding: data-parallel over batch. 8 cores x 2 batches each; weights replicated.

Measured TRN2 facts this design is built on (from NTFF traces of prior
versions): the PE streams 1 moving-column/cycle at 2.4GHz for EVERY dtype --
fp8 DoubleRow only fuses instruction pairs and is net slower per column, so
bf16/fp32r at 1 col/cycle is optimal; a 512-col matmul slot is ~245ns
end-to-end.  ACT exp [128,1024] with bf16 out is ~1110ns; DVE bf16 all-SBUF
adds run ~0.55ns/elem; GPSIMD cannot touch PSUM and its adds are ~2ns/elem.
Engine semaphores are COUNTERS, so any PE wait on engine X transitively
waits for everything queued earlier on X -- cross-engine dependencies must
only point at work that is already drained, or they head-of-line block the
PE.  Exp relief on DVE always lost to this effect; exp is all-ACT.

Per-core design (SPMD, identical program):
  - Token-permuted layout: internal token n~ = j*128 + p maps to real token
    p*16 + j; attention is permutation-equivariant, and this makes the
    x-load / out-store DMAs contiguous per partition.
  - All-bf16 operand path: xT [65, 2048] (bias folded as an all-ones row),
    Q^T/K^T [128d, 2048], V [key, m, d], E [key, m, n] all bf16 (PE speed is
    the same as fp32r; the 2-byte tiles halve SBUF traffic and enable the
    DVE esum below).  End-to-end rel err ~7e-3 vs the 2e-2 gate.
  - The softmax denominator is NOT a third PE stream (that costs ~31us):
    esum[p, n] = sum_m E[p, m, n] is built as a 12+4 split add-tree --
    GPSIMD pair-folds m0-7 early (two 2048-elem bf16 adds), DVE folds m8-11
    mid-window and only m12-15 after the last exp -- then one [1, n]
    ones-matmul pair + transposed reciprocal give 1/den.  Per-element tree
    rounding averages out 1/sqrt(128) in the partition sum.
  - Cross-chunk software pipelining: each (batch, chunk) window runs a
    30-iteration schedule (scores+exp 0-15 with the exp two st-ring slots
    behind, PV lag 2, esum folds at 5/9/13/14/15/17/18/19, den matmul 20,
    den_t + reciprocal and the out-transposes from iter 24 (21 for the last
    window), normalize fused into each PSUM->SBUF copy one iter later,
    stores at +4/+5), and windows start every 20 iterations so one window's
    epilogue always hides under the next window's score stream.  All tail
    instructions are scheduled late enough that their cross-engine inputs
    are already drained when the in-order PE reaches them.
  - PSUM banks (8): score ring 2x(2 banks), PV accumulator 2, den [1,512] 1,
    and one scratch bank holding all 8 output-transpose slots (bf16) with
    den_t borrowing slot 7 under subtile-WAR protection.
"""

import numpy as np

import concourse.bass as bass
import concourse.mybir as mybir
import concourse.tile as tile
from concourse import bacc
from concourse.bass_utils import run_bass_kernel_spmd

N_CORES = 8
B_PER_CORE = 2
N_TOK = 2048
C_IN = 64
D = 128
P = 128
N_TILES = N_TOK // P          # 16
N_CHUNK = 1024
N_CHUNKS = N_TOK // N_CHUNK   # 2
JT = N_CHUNK // P             # 8
SCALE = 1.0 / 8.0             # 1/sqrt(64)

F32 = mybir.dt.float32
BF16 = mybir.dt.bfloat16
I16 = mybir.dt.int16

# Schraudolph constants for exp(S/8) as bf16 bits via int16:
# bits = S * (128*log2e/8) + 128*127 + corrections (+0.5 truncating
# convert, -5.51 centers the piecewise-linear 2^frac sawtooth).
SCH_A = 23.083120654223414
SCH_B = 16256.0 + 0.5 - 0.5 * 128.0 * 0.0861

# exp engine per m (A=ACT exact, D=DVE schraudolph)
EXP_PATTERN = "AAAAAAAAAAAAAAAA"
# relu engine for the 8 projection slices
RELU_PATTERN = "ADADDDDD"

WLEN = 30      # window schedule length (iters)
STRIDE = 20    # window start spacing


def build_program():
    nc = bacc.Bacc("TRN2", target_bir_lowering=False, debug=False,
                   num_devices=N_CORES)

    x = nc.dram_tensor("x", [B_PER_CORE, N_TOK, C_IN], F32, kind="ExternalInput").ap()
    wq = nc.dram_tensor("Wq", [C_IN, D], F32, kind="ExternalInput").ap()
    bq = nc.dram_tensor("bq", [D], F32, kind="ExternalInput").ap()
    wk = nc.dram_tensor("Wk", [C_IN, D], F32, kind="ExternalInput").ap()
    bk = nc.dram_tensor("bk", [D], F32, kind="ExternalInput").ap()
    wv = nc.dram_tensor("Wv", [C_IN, D], F32, kind="ExternalInput").ap()
    bv = nc.dram_tensor("bv", [D], F32, kind="ExternalInput").ap()
    out = nc.dram_tensor("out", [B_PER_CORE, N_TOK, D], F32, kind="ExternalOutput").ap()

    with tile.TileContext(nc) as tc:
        kernel_body(tc, out, x, (wq, bq), (wk, bk), (wv, bv))

    nc.compile()
    return nc


def kernel_body(tc, out, x, qw, kw, vw):
    nc = tc.nc
    from contextlib import ExitStack
    ctx = ExitStack()
    with ctx:
        consts = ctx.enter_context(tc.tile_pool(name="consts", bufs=1))
        perb = ctx.enter_context(tc.tile_pool(name="perb", bufs=2))
        epool = ctx.enter_context(tc.tile_pool(name="epool", bufs=1))
        ep = ctx.enter_context(tc.tile_pool(name="ep", bufs=2))

        # x first: both batches' quarter-DMAs issue before anything else
        x_nat2 = consts.tile([P, N_TILES, B_PER_CORE, C_IN], F32, name="x_nat2",
                             tag="x_nat2")
        H = N_TILES // 4
        for jh in range(4):
            for bb in range(B_PER_CORE):
                eng = nc.sync if bb == 0 else (nc.scalar if jh < 2 else nc.gpsimd)
                eng.dma_start(
                    out=x_nat2[:, jh * H:(jh + 1) * H, bb, :],
                    in_=bass.AP(
                        tensor=x.tensor,
                        offset=bb * N_TOK * C_IN + jh * H * C_IN,
                        ap=[[N_TILES * C_IN, P], [C_IN, H], [1, C_IN]],
                    ),
                )

        # --- constants ---
        identity = consts.tile([P, P], F32)
        nc.vector.memset(identity[:], 0.0)
        nc.gpsimd.affine_select(
            out=identity[:], in_=identity[:],
            compare_op=mybir.AluOpType.not_equal, fill=1.0,
            base=0, pattern=[[-1, P]], channel_multiplier=1)
        id16 = consts.tile([P, P], BF16)
        nc.vector.tensor_copy(out=id16[:], in_=identity[:])
        ones16 = consts.tile([P, 16], BF16)
        nc.vector.memset(ones16[:], 1.0)

        # Bias-folded weights in bf16: w2[c, d], c=64 row is the bias.
        w_sb = {}
        for name, (w, b) in (("q", qw), ("k", kw), ("v", vw)):
            wf = consts.tile([C_IN, D], F32, name=f"wf_{name}", tag=f"wf_{name}")
            nc.sync.dma_start(out=wf[:], in_=w[:])
            bf = consts.tile([1, D], F32, name=f"bf_{name}", tag=f"bf_{name}")
            nc.sync.dma_start(out=bf[:], in_=b[:])
            w2 = consts.tile([C_IN + 1, D], BF16, name=f"w_{name}",
                             tag=f"w_{name}")
            nc.vector.tensor_copy(out=w2[0:C_IN, :], in_=wf[:])
            nc.vector.tensor_copy(out=w2[C_IN:C_IN + 1, :], in_=bf[:])
            w_sb[name] = w2

        xTs = [perb.tile([C_IN + 1, N_TOK], BF16, name=f"xT_{bb}",
                         tag=f"xT_{bb}", bufs=1)
               for bb in range(B_PER_CORE)]
        for bb in range(B_PER_CORE):
            nc.gpsimd.memset(xTs[bb][C_IN:C_IN + 1, :], 1.0)

        qTs = [perb.tile([D, N_TOK], BF16, name=f"qT_{bb}",
                         tag=f"qT_{bb}", bufs=1) for bb in range(B_PER_CORE)]
        kTs = [perb.tile([D, N_TOK], BF16, name=f"kT_{bb}",
                         tag=f"kT_{bb}", bufs=1) for bb in range(B_PER_CORE)]
        v_sbs = [perb.tile([P, N_TILES, D], BF16, name=f"v_sb_{bb}",
                           tag=f"v_sb_{bb}", bufs=1) for bb in range(B_PER_CORE)]

        relu_iter = list(RELU_PATTERN)

        # ---------------- Phase B defs (pools filled in later) ----------------
        pools = {}
        e_all = epool.tile([P, N_TILES, N_CHUNK], BF16, tag="e_all",
                           name="e_all")
        tmpA = epool.tile([P, 4, N_CHUNK], BF16, tag="tmpA", name="tmpA")
        tmpB = epool.tile([P, 4, N_CHUNK], BF16, tag="tmpB", name="tmpB")
        exp_engs = list(EXP_PATTERN)

        windows = [(b, c) for b in range(B_PER_CORE) for c in range(N_CHUNKS)]
        state = {}

        def emit(wi, t):
            b, chunk = windows[wi]
            last = wi == len(windows) - 1
            DEN = 20                    # den matmul iter
            TAIL = 21 if last else 24   # first out-transpose iter
            qT, kT, v_sb = qTs[b], kTs[b], v_sbs[b]
            n0 = chunk * N_CHUNK
            if t == 0:
                state[wi] = {
                    "outu": ep.tile([P, N_CHUNK], BF16, tag="outu",
                                    name=f"outu_{wi}"),
                    "o_sb": ep.tile([P, JT, D], F32, tag="o_sb",
                                    name=f"o_sb_{wi}"),
                    "den_sb": ep.tile([1, N_CHUNK], BF16, tag="den_sb",
                                      name=f"den_sb_{wi}"),
                    "recip": ep.tile([P, JT], F32, tag="recip",
                                     name=f"recip_{wi}"),
                }
            st_ = state[wi]

            if t < N_TILES:
                m = t
                st = pools["pst"].tile([P, N_CHUNK], F32, tag="st",
                                       name=f"st_{wi}_{m}")
                for h in range(2):
                    nc.tensor.matmul(
                        st[:, h * 512:(h + 1) * 512],
                        kT[:, m * P:(m + 1) * P],
                        qT[:, n0 + h * 512:n0 + (h + 1) * 512],
                        start=True, stop=True)
                if exp_engs[m] == "A":
                    nc.scalar.activation(
                        out=e_all[:, m, :], in_=st[:],
                        func=mybir.ActivationFunctionType.Exp, scale=SCALE)
                else:
                    nc.vector.tensor_scalar(
                        out=e_all[:, m, :].bitcast(I16), in0=st[:],
                        scalar1=SCH_A, scalar2=SCH_B,
                        op0=mybir.AluOpType.mult,
                        op1=mybir.AluOpType.add)
            # PV lag 2 (window 0 catches up m=0..6 at t=8, after the
            # prologue's PSUM pools have closed and pacc exists)
            pv_ms = []
            if wi == 0:
                if t == 8:
                    pv_ms = list(range(0, 7))
                elif 9 <= t < N_TILES + 2:
                    pv_ms = [t - 2]
            elif 2 <= t < N_TILES + 2:
                pv_ms = [t - 2]
            for m in pv_ms:
                if m == 0:
                    st_["acc"] = pools["pacc"].tile(
                        [P, N_CHUNK], F32, tag="acc", name=f"acc_{wi}")
                for h in range(2):
                    nc.tensor.matmul(
                        st_["acc"][:, h * 512:(h + 1) * 512],
                        v_sb[:, m, :],
                        e_all[:, m, h * 512:(h + 1) * 512],
                        start=(m == 0), stop=(m == N_TILES - 1))
                if m == N_TILES - 1:
                    nc.vector.tensor_copy(out=st_["outu"][:], in_=st_["acc"][:])
            # esum half-trees (bf16, all-SBUF). A-half (m 0-7) runs early:
            # gpsimd does two 2048-elem pair-adds, DVE folds; B-half after
            # the last exp on DVE.
            if t == 5:
                nc.gpsimd.tensor_tensor(
                    out=tmpA[:, 0:2, :], in0=e_all[:, 0:2, :],
                    in1=e_all[:, 2:4, :], op=mybir.AluOpType.add)
            if t == 9:
                nc.gpsimd.tensor_tensor(
                    out=tmpA[:, 2:4, :], in0=e_all[:, 4:6, :],
                    in1=e_all[:, 6:8, :], op=mybir.AluOpType.add)
            if t == 13:
                # fold m8-11 into tmpB[2:4], collapse tmpA (m0-7)
                nc.vector.tensor_tensor(
                    out=tmpB[:, 2:4, :], in0=e_all[:, 8:10, :],
                    in1=e_all[:, 10:12, :], op=mybir.AluOpType.add)
                nc.vector.tensor_tensor(
                    out=tmpA[:, 0:2, :], in0=tmpA[:, 0:2, :],
                    in1=tmpA[:, 2:4, :], op=mybir.AluOpType.add)
            if t == 14:
                nc.vector.tensor_tensor(
                    out=tmpA[:, 0:1, :], in0=tmpA[:, 0:1, :],
                    in1=tmpA[:, 1:2, :], op=mybir.AluOpType.add)
                nc.vector.tensor_tensor(
                    out=tmpB[:, 2:3, :], in0=tmpB[:, 2:3, :],
                    in1=tmpB[:, 3:4, :], op=mybir.AluOpType.add)
            if t == 15:
                # esum(m0-11) ready mid-window
                nc.vector.tensor_tensor(
                    out=tmpA[:, 0:1, :], in0=tmpA[:, 0:1, :],
                    in1=tmpB[:, 2:3, :], op=mybir.AluOpType.add)
            if t == 17:
                nc.vector.tensor_tensor(
                    out=tmpB[:, 0:2, :], in0=e_all[:, 12:14, :],
                    in1=e_all[:, 14:16, :], op=mybir.AluOpType.add)
            if t == 18:
                nc.vector.tensor_tensor(
                    out=tmpB[:, 0:1, :], in0=tmpB[:, 0:1, :],
                    in1=tmpB[:, 1:2, :], op=mybir.AluOpType.add)
            if t == 19:
                nc.vector.tensor_tensor(
                    out=tmpA[:, 0:1, :], in0=tmpA[:, 0:1, :],
                    in1=tmpB[:, 0:1, :], op=mybir.AluOpType.add)
            if t == DEN:
                # den = ones^T esum, h-halves into sequential 2KB tiles;
                # late enough that the esum merge is surely drained
                for h in range(2):
                    dn = pools["ptr2"].tile([1, 512], F32, tag="den",
                                            name=f"den_{wi}_{h}", bufs=1)
                    nc.tensor.matmul(
                        dn[:], ones16[:, 0:1],
                        tmpA[:, 0, h * 512:(h + 1) * 512],
                        start=True, stop=True)
                    nc.vector.tensor_copy(
                        out=st_["den_sb"][:, h * 512:(h + 1) * 512], in_=dn[:])
            if t == TAIL:
                # one scratch bank (bf16): 8 transpose slots; den_t borrows
                # slot 7 first (subtile WAR keeps tr7 off it until the
                # reciprocal has read)
                scr = pools["ptr2"].tile([P, 8 * P], BF16, tag="scr",
                                         name=f"scr_{wi}", bufs=1)
                st_["scr"] = scr
            if t == TAIL + 1:
                scr = st_["scr"]
                for jt in range(JT):
                    nc.tensor.transpose(
                        scr[:, 7 * P + 2 * jt:7 * P + 2 * jt + 1],
                        st_["den_sb"][:, jt * P:(jt + 1) * P],
                        id16[:1, :1])
                nc.vector.reciprocal(
                    out=st_["recip"][:],
                    in_=scr[:].rearrange("p (a b) -> p a b", b=2)[:, 7 * P // 2:7 * P // 2 + JT, 0])
            # out-transposes 2/iter from TAIL; norms (fused normalize in
            # the PSUM->SBUF copy, DVE) chase one iter behind the reciprocal
            if TAIL <= t < TAIL + 4:
                for jt in range((t - TAIL) * 2, (t - TAIL + 1) * 2):
                    slot = jt * P
                    scr = st_["scr"]
                    nc.tensor.transpose(scr[:, slot:slot + P],
                                        st_["outu"][:, jt * P:(jt + 1) * P],
                                        id16[:])
            if TAIL + 1 <= t < TAIL + 5:
                for jt in range((t - TAIL - 1) * 2, (t - TAIL) * 2):
                    slot = jt * P
                    scr = st_["scr"]
                    if last and jt % 2 == 1:
                        nc.scalar.activation(
                            out=st_["o_sb"][:, jt, :], in_=scr[:, slot:slot + P],
                            func=mybir.ActivationFunctionType.Copy,
                            scale=st_["recip"][:, jt:jt + 1])
                    else:
                        nc.vector.tensor_scalar(
                            out=st_["o_sb"][:, jt, :], in0=scr[:, slot:slot + P],
                            scalar1=st_["recip"][:, jt:jt + 1], scalar2=None,
                            op0=mybir.AluOpType.mult)
            if t == TAIL + 4 or t == TAIL + 5:
                half = t - TAIL - 4
                nc.sync.dma_start(
                    out=bass.AP(
                        tensor=out.tensor,
                        offset=(b * N_TOK + chunk * JT + half * JT // 2) * D,
                        ap=[[N_TILES * D, P], [D, JT // 2], [1, D]],
                    ),
                    in_=st_["o_sb"][:, half * JT // 2:(half + 1) * JT // 2, :],
                )

        # ---------------- Phase A: prologue -----------------------------
        # ppj (4 banks) outlives ptr/pvp (4 banks); once those close, the
        # score ring opens on the RIGHT side of the PSUM heap so window 0's
        # first 8 score iterations overlap the s=1 projection units.
        ppj_cm = tc.tile_pool(name="ppj", bufs=2, space="PSUM")
        ppj = ppj_cm.__enter__()
        with tc.tile_pool(name="ptr", bufs=2, space="PSUM") as ptr, \
             tc.tile_pool(name="pvp", bufs=2, space="PSUM") as pvp:

            def x_tr(q):
                xt4 = ptr.tile([P, 4, P], F32, tag="tr", name=f"xt4_{q}")
                for js in range(4):
                    j = 4 * q + js
                    nc.tensor.transpose(
                        xt4[:, js, :], x_nat2[:, j, :, :], identity[:])
                for bb in range(B_PER_CORE):
                    src = xt4[bb * C_IN:(bb + 1) * C_IN, :, :]
                    dst = xTs[bb][0:C_IN, 4 * q * P:(4 * q + 4) * P]
                    if bb == 0:
                        nc.vector.tensor_copy(
                            out=dst.rearrange("c (j t) -> c j t", j=4), in_=src)
                    else:
                        nc.scalar.copy(
                            out=dst.rearrange("c (j t) -> c j t", j=4), in_=src)

            def v_dir(bb, q):
                vp = pvp.tile([P, 4, P], F32, tag="vp", name=f"vp_{bb}_{q}")
                for js in range(4):
                    j = 4 * q + js
                    nc.tensor.matmul(
                        vp[:, js, :],
                        xTs[bb][:, j * P:(j + 1) * P],
                        w_sb["v"][:],
                        start=True, stop=True)
                dst = v_sbs[bb][:, 4 * q:4 * q + 4, :]
                if bb == 0:
                    nc.vector.tensor_scalar_max(dst, vp[:], 0.0)
                else:
                    nc.scalar.activation(
                        out=dst, in_=vp[:],
                        func=mybir.ActivationFunctionType.Relu, scale=1.0)

            def qk_unit(bb, name, s):
                # projection slice s (1024 tokens)
                t = (qTs if name == "q" else kTs)[bb]
                pj = ppj.tile([D, 2, 512], F32, tag="pj",
                              name=f"pj_{bb}_{name}_{s}")
                for h in range(2):
                    nc.tensor.matmul(
                        pj[:, h, :], w_sb[name][:],
                        xTs[bb][:, s * 1024 + h * 512:s * 1024 + (h + 1) * 512],
                        start=True, stop=True)
                eng = relu_iter.pop(0)
                dst = t[:, s * 1024:(s + 1) * 1024]
                if eng == "A":
                    nc.scalar.activation(
                        out=dst, in_=pj[:],
                        func=mybir.ActivationFunctionType.Relu, scale=1.0)
                else:
                    nc.vector.tensor_scalar_max(dst, pj[:], 0.0)

            for q in range(4):
                x_tr(q)
                if q >= 1:
                    for bb in range(B_PER_CORE):
                        v_dir(bb, q - 1)
                if q == 2:
                    for name in ("q", "k"):
                        for bb in range(B_PER_CORE):
                            qk_unit(bb, name, 0)
            for bb in range(B_PER_CORE):
                v_dir(bb, 3)
        # ptr/pvp closed; right-side score ring + window-0 head vs s=1 units
        pools["pst"] = ctx.enter_context(
            tc.tile_pool(name="pst", bufs=2, space="PSUM", side="right"))
        for i, (name, bb) in enumerate(
                (("q", 0), ("q", 1), ("k", 0), ("k", 1))):
            qk_unit(bb, name, 1)
            emit(0, 2 * i)
            emit(0, 2 * i + 1)
        ppj_cm.__exit__(None, None, None)
        pools["pacc"] = ctx.enter_context(
            tc.tile_pool(name="pacc", bufs=1, space="PSUM"))
        pools["ptr2"] = ctx.enter_context(
            tc.tile_pool(name="ptr2", bufs=2, space="PSUM"))

        starts = [-8] + [12 + STRIDE * i for i in range(len(windows) - 1)]
        total_g = starts[-1] + WLEN
        for g in range(total_g):
            for wi in range(len(windows)):
                t = g - starts[wi]
                if 0 <= t < WLEN and not (wi == 0 and t < 8):
                    emit(wi, t)


_NC_CACHE = None


def _get_program():
    global _NC_CACHE
    if _NC_CACHE is None:
        _NC_CACHE = build_program()
    return _NC_CACHE


def kernel(x, Wq, bq, Wk, bk, Wv, bv, _trace=False):
    x = np.ascontiguousarray(np.asarray(x, dtype=np.float32))
    full_b = x.shape[0]
    assert full_b == N_CORES * B_PER_CORE, x.shape
    nc = _get_program()
    common = {
        "Wq": np.ascontiguousarray(np.asarray(Wq, np.float32)),
        "bq": np.ascontiguousarray(np.asarray(bq, np.float32)),
        "Wk": np.ascontiguousarray(np.asarray(Wk, np.float32)),
        "bk": np.ascontiguousarray(np.asarray(bk, np.float32)),
        "Wv": np.ascontiguousarray(np.asarray(Wv, np.float32)),
        "bv": np.ascontiguousarray(np.asarray(bv, np.float32)),
    }
    in_maps = [
        {"x": x[c * B_PER_CORE:(c + 1) * B_PER_CORE], **common}
        for c in range(N_CORES)
    ]
    res = run_bass_kernel_spmd(nc, in_maps, list(range(N_CORES)), trace=_trace)
    outs = np.concatenate([res.results[c]["out"] for c in range(N_CORES)], axis=0)
    if _trace:
        kernel.last_exec_time_ns = res.exec_time_ns
        kernel.last_trace_info = (res.profile_json,
                                  (res.instructions_and_trace or (None, None))[1])
    return outs
